# revision 1
# baseline (speedup 1.0000x reference)
"""GNN message passing (scatter-add of gathered node features) on 8 TRN2 NeuronCores.

Strategy (edge + node hybrid sharding, no collectives):
  - Outputs are node-sharded: core k owns destination rows [k*12500, (k+1)*12500).
  - Edges are assigned to the core owning their destination row.
  - Per core, each edge is one "token": gather x[col] (one 256B row) from HBM via
    dma_gather into an SBUF message buffer, then accumulate into the core's output
    shard in HBM via dma_scatter_add (SDMA CCE read-modify-write add descriptors).
  - dma_gather indices are int16, so x is addressed in 4 segments of 25000 rows;
    tokens inside each block are grouped by source segment (<=4 sub-gathers/block).
  - Duplicate-destination correctness: concurrent CCE RMW descriptors to the same
    row race (hardware-verified), and per-engine ring order does NOT serialize the
    read-modify-write. Therefore destination rows are UNIQUE within each scatter
    call (each row's edges are dealt to distinct blocks on the host) and scatter
    calls are serialized by waiting each scatter's completion semaphore before
    issuing the next. Rows with more edges than there are main blocks spill into
    extra cleanup blocks. Gathers run pipelined ahead on a separate SWDGE queue.
"""

import numpy as np

# ---- problem constants (hardcoded; must match the harness inputs) ----
N_NODES = 100000
N_EDGES = 1250000
D = 64
NCORES = 8

DEFAULT_PARAMS = dict(
    n_nodes=N_NODES,
    d=D,
    ncores=8,
    shard=12500,      # destination rows per core  (ncores*shard == n_nodes)
    nseg=4,           # x segments for int16 gather indices
    nblk=52,          # main (unique-destination) blocks (~24 chunks each;
                      # >~40-chunk blocks overflow the SWDGE ring and hang)
    nbuf=3,           # message buffers in flight
)


def host_prep(x, edge_index, params=DEFAULT_PARAMS):
    """Deal each destination row's edges across distinct blocks (uniqueness
    within a block), group by source segment within a block, pad to 128-token
    chunks. All cores share one program: per-(block, seg) chunk counts are
    maxed over cores. Returns (per_core_inputs, T, blocks, out_rows, trash)."""
    p = params
    ncores, shard, nseg, nblk = p["ncores"], p["shard"], p["nseg"], p["nblk"]
    segsz = p["n_nodes"] // nseg
    assert nseg * segsz == p["n_nodes"] and ncores * shard == p["n_nodes"]
    trash = shard + (-shard) % 128
    out_rows = trash + 128

    row = np.asarray(edge_index[0], dtype=np.int64)
    col = np.asarray(edge_index[1], dtype=np.int64)

    # ---- per-core edge lists with block assignment ----
    core_of = row // shard
    per_core_edges = []   # (blk, seg, c_loc, r_loc) arrays
    max_k = 0
    for k in range(ncores):
        m = core_of == k
        r = (row[m] - k * shard).astype(np.int64)
        c = col[m]
        order = np.argsort(r, kind="stable")
        r, c = r[order], c[order]
        # rank of each edge within its row group: 0..k_r-1
        grp_start = np.r_[0, np.nonzero(np.diff(r))[0] + 1]
        counts = np.diff(np.r_[grp_start, len(r)])
        max_k = max(max_k, int(counts.max()) if len(counts) else 0)
        rank = np.arange(len(r)) - np.repeat(grp_start, counts)
        # pseudo-random per-row start offset for balance
        h = (r * 2654435761) % nblk
        blk = (np.repeat(h[grp_start], counts) + rank)  # rank < nblk -> main
        seg = c // segsz
        per_core_edges.append((blk, rank, seg,
                               (c - seg * segsz).astype(np.int16),
                               r.astype(np.int16)))

    n_clean = max(2, max_k - nblk)   # cleanup blocks for spilled ranks
    nblk_tot = nblk + n_clean

    # resolve final block id (main: (h+rank) % nblk ; spill: nblk + (rank-nblk))
    counts_bs = np.zeros((ncores, nblk_tot, nseg), dtype=np.int64)
    resolved = []
    for k in range(ncores):
        blk, rank, seg, c_loc, r_loc = per_core_edges[k]
        main = rank < nblk
        b = np.where(main, blk % nblk, nblk + (rank - nblk))
        assert b.max(initial=0) < nblk_tot
        np.add.at(counts_bs[k], (b, seg), 1)
        resolved.append((b, seg, c_loc, r_loc))

    # per-(block, seg) chunk counts, shared across cores
    chunks_bs = -(np.max(counts_bs, axis=0) // -128)   # [nblk_tot, nseg]
    tok_bs = chunks_bs * 128
    # token offset of each (block, seg) group in the global stream
    off_bs = np.zeros_like(tok_bs)
    off = 0
    blocks = []   # per block: (tok0, ntok, [(seg, sub_tok0, nchunks), ...])
    for b in range(nblk_tot):
        tok0 = off
        subs = []
        for s in range(nseg):
            off_bs[b, s] = off
            if chunks_bs[b, s] > 0:
                subs.append((s, off, int(chunks_bs[b, s])))
            off += int(tok_bs[b, s])
        ntok = off - tok0
        if ntok > 0:
            blocks.append((tok0, ntok, subs))
    T = off
    assert T % 128 == 0

    per_core = []
    x = np.asarray(x, dtype=np.float32)
    for k in range(ncores):
        b, seg, c_loc, r_loc = resolved[k]
        gidx = np.zeros(T, dtype=np.int16)          # pad gathers read x_seg[0]
        sidx = np.full(T, trash, dtype=np.int16)    # pad scatters hit trash row
        # position within each (b, seg) cell
        order = np.lexsort((seg, b))
        bs_sorted = b[order] * nseg + seg[order]
        starts = np.r_[0, np.nonzero(np.diff(bs_sorted))[0] + 1]
        cnts = np.diff(np.r_[starts, len(bs_sorted)])
        within = np.arange(len(bs_sorted)) - np.repeat(starts, cnts)
        tok = off_bs[b[order], seg[order]] + within
        gidx[tok] = c_loc[order]
        sidx[tok] = r_loc[order]
        gw = np.tile(gidx.reshape(-1, 16).T, (8, 1)).copy()
        sw = np.tile(sidx.reshape(-1, 16).T, (8, 1)).copy()
        per_core.append({"x": x, "gidx": gw, "sidx": sw})

    return per_core, T, blocks, out_rows, trash


def build_bass(T, blocks, params=DEFAULT_PARAMS, out_rows=None):
    import concourse.bacc as bacc
    import concourse.mybir as mybir
    import contextlib

    p = params
    d, nseg, nbuf = p["d"], p["nseg"], p["nbuf"]
    segsz = p["n_nodes"] // nseg

    nc = bacc.Bacc(
        None, target_bir_lowering=False, debug=False, num_swdge_queues=2
    )
    x = nc.dram_tensor("x", [p["n_nodes"], d], mybir.dt.float32, kind="ExternalInput")
    gidx = nc.dram_tensor("gidx", [128, T // 16], mybir.dt.int16, kind="ExternalInput")
    sidx = nc.dram_tensor("sidx", [128, T // 16], mybir.dt.int16, kind="ExternalInput")
    out = nc.dram_tensor("out", [out_rows, d], mybir.dt.float32, kind="ExternalOutput")

    NB = len(blocks)
    max_chunks = max(ntok for _, ntok, _ in blocks) // 128
    # cap tokens per DMA call so its descriptor stream fits the SWDGE ring
    # (~256 descs per engine lane; scatter tx pushes ~ntok/8 per lane)
    cap_ch = 15
    # cumulative sub-gather count per buffer slot, for exact gsem waits
    gcnt = [0] * nbuf
    scnt = [0]  # cumulative scatter call count

    with (
        nc.sbuf_tensor([128, T // 16], mybir.dt.int16) as gi_sb,
        nc.sbuf_tensor([128, T // 16], mybir.dt.int16) as si_sb,
        nc.sbuf_tensor([128, nbuf * max_chunks * d], mybir.dt.float32) as msg,
        nc.semaphore("lsem") as lsem,
        nc.semaphore("ssem") as ssem,
        contextlib.ExitStack() as stack,
        nc.Block() as block,
    ):
        gsems = [stack.enter_context(nc.semaphore(f"gsem{i}")) for i in range(nbuf)]

        @block.gpsimd
        def _(g):
            g.dma_start(out=gi_sb[:], in_=gidx[:]).then_inc(lsem, 16)
            g.dma_start(out=si_sb[:], in_=sidx[:]).then_inc(lsem, 16)
            g.wait_ge(lsem, 32)

            def gathers(j):
                tok0, ntok, subs = blocks[j]
                i = j % nbuf
                base = i * max_chunks * d
                for s, sub0, nch in subs:
                    for c0 in range(0, nch, cap_ch):
                        cc = min(cap_ch, nch - c0)
                        p0 = sub0 + c0 * 128
                        boff = base + ((sub0 - tok0) // 128 + c0) * d
                        buf = msg[:, boff:boff + cc * d]
                        g.dma_gather(
                            out_ap=buf.rearrange("p (k dd) -> p k dd", dd=d),
                            in_ap=x[s * segsz:(s + 1) * segsz, :],
                            idxs_ap=gi_sb[:, p0 // 16:(p0 + cc * 128) // 16],
                            num_idxs=cc * 128,
                            num_idxs_reg=cc * 128,
                            elem_size=d,
                            queue_num=1,
                        ).then_inc(gsems[i], 16)
                        gcnt[i] += 1

            def scatter(b):
                tok0, ntok, _ = blocks[b]
                i = b % nbuf
                base = i * max_chunks * d
                g.wait_ge(gsems[i], 16 * gcnt[i])   # all sub-gathers of block b
                nch = ntok // 128
                for c0 in range(0, nch, cap_ch):
                    cc = min(cap_ch, nch - c0)
                    p0 = tok0 + c0 * 128
                    g.dma_scatter_add(
                        out_ap=out[:],
                        in_ap=msg[:, base + c0 * d:base + (c0 + cc) * d].rearrange(
                            "p (k dd) -> p k dd", dd=d),
                        idxs_ap=si_sb[:, p0 // 16:(p0 + cc * 128) // 16],
                        num_idxs=cc * 128,
                        num_idxs_reg=cc * 128,
                        elem_size=d,
                        queue_num=0,
                    ).then_inc(ssem, 16)
                    scnt[0] += 1

            for j in range(min(nbuf - 1, NB)):
                gathers(j)
            for b in range(NB):
                # serialize between blocks: all of block b-1's scatters fully
                # landed (also frees the buffer slot gathers b+nbuf-1 reuse)
                if b > 0:
                    g.wait_ge(ssem, 16 * scnt[0])
                jg = b + nbuf - 1
                if jg < NB:
                    gathers(jg)
                scatter(b)
            g.wait_ge(ssem, 16 * scnt[0])

    nc.compile()
    return nc


def run_spmd(nc, per_core, trace=False):
    from concourse.bass_utils import run_bass_kernel_spmd
    return run_bass_kernel_spmd(
        nc, per_core, core_ids=list(range(len(per_core))), trace=trace
    )


def kernel(x, edge_index, _trace=False, _return_results=False):
    x = np.asarray(x, dtype=np.float32)
    params = DEFAULT_PARAMS
    per_core, T, blocks, out_rows, trash = host_prep(x, edge_index, params)
    nc = build_bass(T, blocks, params, out_rows)
    res = run_spmd(nc, per_core, trace=_trace)
    shard = params["shard"]
    out = np.concatenate(
        [res.results[k]["out"][:shard] for k in range(params["ncores"])], axis=0)
    if _return_results:
        return out, res
    return out



# revision 12
# speedup vs baseline: 2.1804x; 2.1804x over previous
"""GNN message passing (scatter-add of gathered node features) on 8 TRN2 NeuronCores.

Strategy (node-sharded outputs, no collectives, no HBM read-modify-write):
  - Core k owns destination rows [k*12500, (k+1)*12500); its edges are those
    whose dest row lands in the shard. Output rows are grouped into 98 blocks
    of 128 rows.
  - Host sorts each core's edges by (block, source-segment) into fixed 128-token
    chunks; per-(block,seg) "cells" are sized by the max count over cores so all
    8 cores share one program. Padding tokens carry gather idx -1 (SWDGE skips
    trailing negatives; per-core exact counts come from a register loaded from
    an uploaded per-cell count table) and one-hot slot 255 (contributes zero).
  - Per chunk: dma_gather pulls 128 x-rows (256B each) from HBM into SBUF;
    scalar engine converts fp32->bf16; vector engine builds a [128 tok x 128
    slot] bf16 one-hot by comparing the per-token slot id against an iota row;
    PE matmul accumulates one-hot^T @ msg into the block's PSUM tile
    (start/stop over the block's chunks). This replaces the serialized CCE
    scatter-add entirely.
  - Finished PSUM blocks are copied to an SBUF stage (vector) and DMA'd to the
    output shard in HBM (sync engine HWDGE). Everything is ring-buffered and
    pipelined; gathers run on 2 SWDGE queues.
"""

import numpy as np

N_NODES = 100000
N_EDGES = 1250000
D = 64
NCORES = 8
SHARD = 12500
NBLK = 98              # ceil(12500/128); last block has 84 valid rows
NSEG = 4               # int16 gather indices -> x addressed in 4 segments
SEGSZ = 25000
NCELL = NBLK * NSEG
OUT_ROWS = NBLK * 128  # 12544

# pipeline depths
RC = 24      # msg ring, in cells
SEL = 64     # one-hot ring, in chunks
G = 8        # chunks per DVE compare instruction
NPS = 8      # psum tiles (one bank each)
NSTG = 4     # output stage ring, in blocks


def host_prep(x, edge_index):
    row = np.asarray(edge_index[0], dtype=np.int64)
    col = np.asarray(edge_index[1], dtype=np.int64)
    core = row // SHARD
    rloc = row - core * SHARD
    blk = rloc >> 7
    slot = rloc & 127
    seg = col // SEGSZ
    cloc = (col - seg * SEGSZ).astype(np.int16)
    cell = blk * NSEG + seg

    counts = np.zeros((NCORES, NCELL), dtype=np.int64)
    np.add.at(counts, (core, cell), 1)
    cnt_eff = np.maximum(counts, 1)          # empty cells get one dummy token
    nch = -(np.max(cnt_eff, axis=0) // -128)  # chunks per cell, shared
    cum = np.concatenate([[0], np.cumsum(nch)]).astype(np.int64)
    TC = int(cum[-1])
    T = TC * 128
    off = cum[:-1] * 128                     # token offset per cell

    import ml_dtypes
    per_core = []
    x = np.ascontiguousarray(np.asarray(x, dtype=np.float32))
    for k in range(NCORES):
        m = core == k
        ck = cell[m]
        order = np.argsort(ck, kind="stable")
        cc = ck[order]
        cl = cloc[m][order]
        sl = slot[m][order]
        cnts = np.bincount(cc, minlength=NCELL)
        starts = np.concatenate([[0], np.cumsum(cnts)])[:-1]
        within = np.arange(len(cc)) - starts[cc]
        tok = off[cc] + within
        gidx = np.full(T, -1, dtype=np.int16)
        slot_arr = np.full(T, 255, dtype=np.int32)
        gidx[tok] = cl
        slot_arr[tok] = sl
        empty = cnts == 0
        gidx[off[empty]] = 0                  # dummy valid token, slot stays 255
        gw = np.tile(gidx.reshape(-1, 16).T, (8, 1)).copy()
        sw = np.ascontiguousarray(
            slot_arr.reshape(TC, 128).T.astype(ml_dtypes.bfloat16))
        cntk = np.broadcast_to(
            np.maximum(cnts, 1).astype(np.int32), (128, NCELL)).copy()
        per_core.append({"x": x, "gidx": gw, "slot": sw, "cnt": cntk})

    return per_core, nch, cum, T, TC


def build_bass(nch, cum, T, TC):
    import concourse.bacc as bacc
    import concourse.mybir as mybir
    from concourse.bass import AP
    import contextlib

    f32, bf16, i16, i32 = (mybir.dt.float32, mybir.dt.bfloat16,
                           mybir.dt.int16, mybir.dt.int32)

    maxnch = int(np.max(nch))
    # cells: (j, seg, nch_j, first_chunk, tok0, ring_col)
    cells = []
    for j in range(NCELL):
        cells.append((j, j % NSEG, int(nch[j]), int(cum[j]), int(cum[j]) * 128,
                      (j % RC) * maxnch * 64))
    chunk_end = [int(cum[j + 1]) for j in range(NCELL)]  # chunks through cell j
    blk_chunk_end = [int(cum[(b + 1) * NSEG]) for b in range(NBLK)]

    nc = bacc.Bacc(None, target_bir_lowering=False, debug=False,
                   num_swdge_queues=2)
    x = nc.dram_tensor("x", [N_NODES, D], f32, kind="ExternalInput")
    gidx = nc.dram_tensor("gidx", [128, T // 16], i16, kind="ExternalInput")
    slot = nc.dram_tensor("slot", [128, TC], bf16, kind="ExternalInput")
    cnt = nc.dram_tensor("cnt", [128, NCELL], i32, kind="ExternalInput")
    out = nc.dram_tensor("out", [OUT_ROWS, D], f32, kind="ExternalOutput")

    last_wait = {}

    def wge(eng, sem, val):
        if val <= 0:
            return
        key = (id(eng), id(sem))
        if last_wait.get(key, 0) >= val:
            return
        eng.wait_ge(sem, val)
        last_wait[key] = val

    with (
        nc.sbuf_tensor([128, T // 16], i16) as gi_sb,
        nc.sbuf_tensor([128, TC], bf16) as slot_sb,
        nc.sbuf_tensor([128, NCELL], i32) as cnt_sb,
        nc.sbuf_tensor([128, 128], bf16) as iota_sb,
        nc.sbuf_tensor([128, RC * maxnch * 64], f32) as msg32,
        nc.sbuf_tensor([128, RC * maxnch * 64], bf16) as msg16,
        nc.sbuf_tensor([128, SEL * 128], bf16) as selT,
        nc.sbuf_tensor([128, NSTG * 64], f32) as stage,
        nc.semaphore("lsem") as lsem,
        nc.semaphore("msem") as msem,
        nc.semaphore("isem") as isem,
        nc.semaphore("csem") as csem,
        nc.semaphore("vsem") as vsem,
        nc.semaphore("pesem") as pesem,
        nc.semaphore("cpsem") as cpsem,
        contextlib.ExitStack() as stack,
        nc.Block() as block,
    ):
        # DMA completion sems rotate as deep as the consumer ring so a sem's
        # previous +16 is always consumed before its next DMA issues (the
        # sim's race detector rejects concurrent increments on one sem).
        gsems = [stack.enter_context(nc.semaphore(f"gsem{i}"))
                 for i in range(RC)]
        osems = [stack.enter_context(nc.semaphore(f"osem{i}"))
                 for i in range(NSTG)]
        psums = [stack.enter_context(nc.psum_tensor(f"ps{i}", [128, 64], f32))
                 for i in range(NPS)]

        @block.gpsimd
        def _(g):
            g.iota(iota_sb[:, :], [[1, 128]], channel_multiplier=0,
                   allow_small_or_imprecise_dtypes=True).then_inc(isem, 1)
            g.dma_start(out=gi_sb[:], in_=gidx[:]).then_inc(lsem, 16)
            g.dma_start(out=slot_sb[:], in_=slot[:]).then_inc(lsem, 16)
            g.dma_start(out=cnt_sb[:], in_=cnt[:]).then_inc(lsem, 16)
            g.wait_ge(lsem, 48)
            g.wait_ge(msem, 1)
            with g.register("gr") as gr:
                for j, s, nch_j, fc, tok0, rcol in cells:
                    wge(g, csem, j - RC + 1)
                    g.reg_load(gr, cnt_sb[0:1, j:j + 1])
                    buf = msg32[:, rcol:rcol + nch_j * 64]
                    g.dma_gather(
                        out_ap=buf.rearrange("p (k dd) -> p k dd", dd=D),
                        in_ap=x[s * SEGSZ:(s + 1) * SEGSZ, :],
                        idxs_ap=gi_sb[:, tok0 // 16:(tok0 + nch_j * 128) // 16],
                        num_idxs=nch_j * 128,
                        num_idxs_reg=gr,
                        elem_size=D,
                        queue_num=j % 2,
                    ).then_inc(gsems[j % RC], 16)

        @block.scalar
        def _(se):
            se.wait_ge(msem, 1)
            for j, s, nch_j, fc, tok0, rcol in cells:
                wge(se, gsems[j % RC], 16 * (j // RC + 1))
                if j >= RC:
                    wge(se, pesem, chunk_end[j - RC])
                se.copy(out=msg16[:, rcol:rcol + nch_j * 64],
                        in_=msg32[:, rcol:rcol + nch_j * 64]).then_inc(csem)

        @block.vector
        def _(ve):
            ve.memset(msg32[:], 0).then_inc(msem, 1)
            ve.wait_ge(lsem, 48)
            ve.wait_ge(isem, 1)
            ngroups = -(TC // -G)
            # merge compare groups and psum->stage copies in issue order
            events = []
            for gidx_ in range(ngroups):
                events.append((gidx_, 0, "cmp", gidx_))
            for b in range(NBLK):
                gb = (blk_chunk_end[b] - 1) // G
                events.append((gb, 1, "copy", b))
            events.sort(key=lambda e: (e[0], e[1]))
            for _, _, kind, v in events:
                if kind == "cmp":
                    g0 = v * G
                    gg = min(G, TC - g0)
                    wge(ve, pesem, g0 + gg - SEL)
                    out_ap = AP(selT, (g0 % SEL) * 128,
                                [[SEL * 128, 128], [128, gg], [1, 128]])
                    in0 = AP(slot_sb, g0,
                             [[TC, 128], [1, gg], [0, 128]])
                    in1 = AP(iota_sb, 0,
                             [[128, 128], [0, gg], [1, 128]])
                    ve.tensor_tensor(out_ap, in0, in1,
                                     mybir.AluOpType.is_equal).then_inc(vsem)
                else:
                    b = v
                    wge(ve, pesem, blk_chunk_end[b])
                    if b >= NSTG:
                        wge(ve, osems[b % NSTG], 16 * (b // NSTG))
                    ve.tensor_copy(out=stage[:, (b % NSTG) * 64:(b % NSTG + 1) * 64],
                                   in_=psums[b % NPS][:, :]).then_inc(cpsem)

        @block.tensor
        def _(te):
            for b in range(NBLK):
                first_c = blk_chunk_end[b - 1] if b > 0 else 0
                last_c = blk_chunk_end[b] - 1
                for j in range(b * NSEG, (b + 1) * NSEG):
                    _, s, nch_j, fc, tok0, rcol = cells[j]
                    wge(te, csem, j + 1)
                    for ci in range(nch_j):
                        c = fc + ci
                        wge(te, vsem, c // G + 1)
                        if c == first_c:
                            wge(te, cpsem, b - NPS + 1)
                        te.matmul(
                            psums[b % NPS][:, :],
                            selT[:, (c % SEL) * 128:(c % SEL + 1) * 128],
                            msg16[:, rcol + ci * 64:rcol + (ci + 1) * 64],
                            start=(c == first_c),
                            stop=(c == last_c),
                        ).then_inc(pesem)

        @block.sync
        def _(sy):
            for b in range(NBLK):
                wge(sy, cpsem, b + 1)
                sy.dma_start(
                    out=out[b * 128:(b + 1) * 128, :],
                    in_=stage[:, (b % NSTG) * 64:(b % NSTG + 1) * 64],
                ).then_inc(osems[b % NSTG], 16)
            for i in range(NSTG):
                n_i = NBLK // NSTG + (1 if i < NBLK % NSTG else 0)
                sy.wait_ge(osems[i], 16 * n_i)

    nc.compile()
    return nc


def run_spmd(nc, per_core, trace=False):
    from concourse.bass_utils import run_bass_kernel_spmd
    return run_bass_kernel_spmd(
        nc, per_core, core_ids=list(range(len(per_core))), trace=trace
    )


def kernel(x, edge_index, _trace=False, _return_results=False):
    x = np.asarray(x, dtype=np.float32)
    per_core, nch, cum, T, TC = host_prep(x, edge_index)
    nc = build_bass(nch, cum, T, TC)
    res = run_spmd(nc, per_core, trace=_trace)
    out = np.concatenate(
        [res.results[k]["out"][:SHARD] for k in range(NCORES)], axis=0)
    if _return_results:
        return out, res
    return out


# revision 14
# speedup vs baseline: 4.6446x; 2.1301x over previous
"""GNN message passing (scatter-add of gathered node features) on 8 TRN2 NeuronCores.

Strategy (node-sharded outputs, no collectives, no HBM read-modify-write):
  - Core k owns destination rows [k*12500, (k+1)*12500); its edges are those
    whose dest row lands in the shard. Output rows are grouped into 98 blocks
    of 128 rows.
  - Host sorts each core's edges by (block, source-segment) into fixed 128-token
    chunks; per-(block,seg) "cells" are sized by the max count over cores so all
    8 cores share one program. Padding tokens carry gather idx -1 (SWDGE skips
    trailing negatives; per-core exact counts come from a register loaded from
    an uploaded per-cell count table) and one-hot slot 255 (contributes zero).
  - Per chunk: dma_gather pulls 128 x-rows (256B each) from HBM into SBUF;
    scalar engine converts fp32->bf16; vector engine builds a [128 tok x 128
    slot] bf16 one-hot by comparing the per-token slot id against an iota row;
    PE matmul accumulates one-hot^T @ msg into the block's PSUM tile
    (start/stop over the block's chunks). This replaces the serialized CCE
    scatter-add entirely.
  - Finished PSUM blocks are copied to an SBUF stage (vector) and DMA'd to the
    output shard in HBM (sync engine HWDGE). Everything is ring-buffered and
    pipelined; gathers run on 2 SWDGE queues.
"""

import numpy as np

N_NODES = 100000
N_EDGES = 1250000
D = 64
NCORES = 8
SHARD = 12500
NBLK = 98              # ceil(12500/128); last block has 84 valid rows
NSEG = 4               # int16 gather indices -> x addressed in 4 segments
SEGSZ = 25000
NCELL = NBLK * NSEG
OUT_ROWS = NBLK * 128  # 12544

# pipeline depths
RC = 24      # msg ring, in cells
SEL = 64     # one-hot ring, in chunks
G = 8        # chunks per DVE compare instruction
NPS = 8      # psum tiles (one bank each)
NSTG = 4     # output stage ring, in blocks


def host_prep(x, edge_index):
    row = np.asarray(edge_index[0], dtype=np.int64)
    col = np.asarray(edge_index[1], dtype=np.int64)
    core = row // SHARD
    rloc = row - core * SHARD
    blk = rloc >> 7
    slot = rloc & 127
    seg = col // SEGSZ
    cloc = (col - seg * SEGSZ).astype(np.int16)
    cell = blk * NSEG + seg

    counts = np.zeros((NCORES, NCELL), dtype=np.int64)
    np.add.at(counts, (core, cell), 1)
    cnt_eff = np.maximum(counts, 1)          # empty cells get one dummy token
    nch = -(np.max(cnt_eff, axis=0) // -128)  # chunks per cell, shared
    cum = np.concatenate([[0], np.cumsum(nch)]).astype(np.int64)
    TC = int(cum[-1])
    T = TC * 128
    off = cum[:-1] * 128                     # token offset per cell

    import ml_dtypes
    per_core = []
    x = np.ascontiguousarray(np.asarray(x, dtype=np.float32))
    for k in range(NCORES):
        m = core == k
        ck = cell[m]
        order = np.argsort(ck, kind="stable")
        cc = ck[order]
        cl = cloc[m][order]
        sl = slot[m][order]
        cnts = np.bincount(cc, minlength=NCELL)
        starts = np.concatenate([[0], np.cumsum(cnts)])[:-1]
        within = np.arange(len(cc)) - starts[cc]
        tok = off[cc] + within
        gidx = np.full(T, -1, dtype=np.int16)
        slot_arr = np.full(T, 255, dtype=np.int32)
        gidx[tok] = cl
        slot_arr[tok] = sl
        empty = cnts == 0
        gidx[off[empty]] = 0                  # dummy valid token, slot stays 255
        gw = np.tile(gidx.reshape(-1, 16).T, (8, 1)).copy()
        sw = np.ascontiguousarray(
            slot_arr.reshape(TC, 128).T.astype(ml_dtypes.bfloat16))
        cntk = np.broadcast_to(
            np.maximum(cnts, 1).astype(np.int32), (128, NCELL)).copy()
        per_core.append({"x": x, "gidx": gw, "slot": sw, "cnt": cntk})

    return per_core, nch, cum, T, TC


def build_bass(nch, cum, T, TC):
    import concourse.bacc as bacc
    import concourse.mybir as mybir
    from concourse.bass import AP
    import contextlib

    f32, bf16, i16, i32 = (mybir.dt.float32, mybir.dt.bfloat16,
                           mybir.dt.int16, mybir.dt.int32)

    maxnch = int(np.max(nch))
    # cells: (j, seg, nch_j, first_chunk, tok0, ring_col)
    cells = []
    for j in range(NCELL):
        cells.append((j, j % NSEG, int(nch[j]), int(cum[j]), int(cum[j]) * 128,
                      (j % RC) * maxnch * 64))
    chunk_end = [int(cum[j + 1]) for j in range(NCELL)]  # chunks through cell j
    blk_chunk_end = [int(cum[(b + 1) * NSEG]) for b in range(NBLK)]

    nc = bacc.Bacc(None, target_bir_lowering=False, debug=False,
                   num_swdge_queues=4)
    x = nc.dram_tensor("x", [N_NODES, D], f32, kind="ExternalInput")
    gidx = nc.dram_tensor("gidx", [128, T // 16], i16, kind="ExternalInput")
    slot = nc.dram_tensor("slot", [128, TC], bf16, kind="ExternalInput")
    cnt = nc.dram_tensor("cnt", [128, NCELL], i32, kind="ExternalInput")
    out = nc.dram_tensor("out", [OUT_ROWS, D], f32, kind="ExternalOutput")

    last_wait = {}

    def wge(eng, sem, val):
        if val <= 0:
            return
        key = (id(eng), id(sem))
        if last_wait.get(key, 0) >= val:
            return
        eng.wait_ge(sem, val)
        last_wait[key] = val

    with (
        nc.sbuf_tensor([128, T // 16], i16) as gi_sb,
        nc.sbuf_tensor([128, TC], bf16) as slot_sb,
        nc.sbuf_tensor([128, NCELL], i32) as cnt_sb,
        nc.sbuf_tensor([128, 128], bf16) as iota_sb,
        nc.sbuf_tensor([128, RC * maxnch * 64], f32) as msg32,
        nc.sbuf_tensor([128, RC * maxnch * 64], bf16) as msg16,
        nc.sbuf_tensor([128, SEL * 128], bf16) as selT,
        nc.sbuf_tensor([128, NSTG * 64], f32) as stage,
        nc.semaphore("lsem") as lsem,
        nc.semaphore("msem") as msem,
        nc.semaphore("isem") as isem,
        nc.semaphore("csem") as csem,
        nc.semaphore("vsem") as vsem,
        nc.semaphore("pesem") as pesem,
        nc.semaphore("cpsem") as cpsem,
        contextlib.ExitStack() as stack,
        nc.Block() as block,
    ):
        # DMA completion sems rotate as deep as the consumer ring so a sem's
        # previous +16 is always consumed before its next DMA issues (the
        # sim's race detector rejects concurrent increments on one sem).
        gsems = [stack.enter_context(nc.semaphore(f"gsem{i}"))
                 for i in range(RC)]
        osems = [stack.enter_context(nc.semaphore(f"osem{i}"))
                 for i in range(NSTG)]
        psums = [stack.enter_context(nc.psum_tensor(f"ps{i}", [128, 64], f32))
                 for i in range(NPS)]

        @block.gpsimd
        def _(g):
            g.iota(iota_sb[:, :], [[1, 128]], channel_multiplier=0,
                   allow_small_or_imprecise_dtypes=True).then_inc(isem, 1)
            g.dma_start(out=gi_sb[:], in_=gidx[:]).then_inc(lsem, 16)
            g.dma_start(out=slot_sb[:], in_=slot[:]).then_inc(lsem, 16)
            g.dma_start(out=cnt_sb[:], in_=cnt[:]).then_inc(lsem, 16)
            g.wait_ge(lsem, 48)
            g.wait_ge(msem, 1)
            with g.register("gr") as gr:
                for j, s, nch_j, fc, tok0, rcol in cells:
                    wge(g, csem, j - RC + 1)
                    g.reg_load(gr, cnt_sb[0:1, j:j + 1])
                    buf = msg32[:, rcol:rcol + nch_j * 64]
                    g.dma_gather(
                        out_ap=buf.rearrange("p (k dd) -> p k dd", dd=D),
                        in_ap=x[s * SEGSZ:(s + 1) * SEGSZ, :],
                        idxs_ap=gi_sb[:, tok0 // 16:(tok0 + nch_j * 128) // 16],
                        num_idxs=nch_j * 128,
                        num_idxs_reg=gr,
                        elem_size=D,
                        queue_num=j % 4,
                    ).then_inc(gsems[j % RC], 16)

        @block.scalar
        def _(se):
            se.wait_ge(msem, 1)
            for j, s, nch_j, fc, tok0, rcol in cells:
                wge(se, gsems[j % RC], 16 * (j // RC + 1))
                if j >= RC:
                    wge(se, pesem, chunk_end[j - RC])
                se.copy(out=msg16[:, rcol:rcol + nch_j * 64],
                        in_=msg32[:, rcol:rcol + nch_j * 64]).then_inc(csem)

        @block.vector
        def _(ve):
            ve.memset(msg32[:], 0).then_inc(msem, 1)
            ve.wait_ge(lsem, 48)
            ve.wait_ge(isem, 1)
            ngroups = -(TC // -G)
            # merge compare groups and psum->stage copies in issue order
            events = []
            for gidx_ in range(ngroups):
                events.append((gidx_, 0, "cmp", gidx_))
            for b in range(NBLK):
                gb = (blk_chunk_end[b] - 1) // G
                events.append((gb, 1, "copy", b))
            events.sort(key=lambda e: (e[0], e[1]))
            for _, _, kind, v in events:
                if kind == "cmp":
                    g0 = v * G
                    gg = min(G, TC - g0)
                    wge(ve, pesem, g0 + gg - SEL)
                    out_ap = AP(selT, (g0 % SEL) * 128,
                                [[SEL * 128, 128], [128, gg], [1, 128]])
                    in0 = AP(slot_sb, g0,
                             [[TC, 128], [1, gg], [0, 128]])
                    in1 = AP(iota_sb, 0,
                             [[128, 128], [0, gg], [1, 128]])
                    ve.tensor_tensor(out_ap, in0, in1,
                                     mybir.AluOpType.is_equal).then_inc(vsem)
                else:
                    b = v
                    wge(ve, pesem, blk_chunk_end[b])
                    if b >= NSTG:
                        wge(ve, osems[b % NSTG], 16 * (b // NSTG))
                    ve.tensor_copy(out=stage[:, (b % NSTG) * 64:(b % NSTG + 1) * 64],
                                   in_=psums[b % NPS][:, :]).then_inc(cpsem)

        @block.tensor
        def _(te):
            for b in range(NBLK):
                first_c = blk_chunk_end[b - 1] if b > 0 else 0
                last_c = blk_chunk_end[b] - 1
                for j in range(b * NSEG, (b + 1) * NSEG):
                    _, s, nch_j, fc, tok0, rcol = cells[j]
                    wge(te, csem, j + 1)
                    for ci in range(nch_j):
                        c = fc + ci
                        wge(te, vsem, c // G + 1)
                        if c == first_c:
                            wge(te, cpsem, b - NPS + 1)
                        te.matmul(
                            psums[b % NPS][:, :],
                            selT[:, (c % SEL) * 128:(c % SEL + 1) * 128],
                            msg16[:, rcol + ci * 64:rcol + (ci + 1) * 64],
                            start=(c == first_c),
                            stop=(c == last_c),
                        ).then_inc(pesem)

        @block.sync
        def _(sy):
            for b in range(NBLK):
                wge(sy, cpsem, b + 1)
                sy.dma_start(
                    out=out[b * 128:(b + 1) * 128, :],
                    in_=stage[:, (b % NSTG) * 64:(b % NSTG + 1) * 64],
                ).then_inc(osems[b % NSTG], 16)
            for i in range(NSTG):
                n_i = NBLK // NSTG + (1 if i < NBLK % NSTG else 0)
                sy.wait_ge(osems[i], 16 * n_i)

    nc.compile()
    return nc


def run_spmd(nc, per_core, trace=False):
    from concourse.bass_utils import run_bass_kernel_spmd
    return run_bass_kernel_spmd(
        nc, per_core, core_ids=list(range(len(per_core))), trace=trace
    )


def kernel(x, edge_index, _trace=False, _return_results=False):
    x = np.asarray(x, dtype=np.float32)
    per_core, nch, cum, T, TC = host_prep(x, edge_index)
    nc = build_bass(nch, cum, T, TC)
    res = run_spmd(nc, per_core, trace=_trace)
    out = np.concatenate(
        [res.results[k]["out"][:SHARD] for k in range(NCORES)], axis=0)
    if _return_results:
        return out, res
    return out


# revision 15
# speedup vs baseline: 4.7317x; 1.0188x over previous
"""GNN message passing (scatter-add of gathered node features) on 8 TRN2 NeuronCores.

Strategy (node-sharded outputs, no collectives, no HBM read-modify-write):
  - Core k owns destination rows [k*12500, (k+1)*12500); its edges are those
    whose dest row lands in the shard. Output rows are grouped into 98 blocks
    of 128 rows.
  - Host sorts each core's edges by (block, source-segment) into fixed 128-token
    chunks; per-(block,seg) "cells" are sized by the max count over cores so all
    8 cores share one program. Padding tokens carry gather idx -1 (SWDGE skips
    trailing negatives; per-core exact counts come from a register loaded from
    an uploaded per-cell count table) and one-hot slot 255 (contributes zero).
  - Per chunk: dma_gather pulls 128 x-rows (256B each) from HBM into SBUF;
    scalar engine converts fp32->bf16; vector engine builds a [128 tok x 128
    slot] bf16 one-hot by comparing the per-token slot id against an iota row;
    PE matmul accumulates one-hot^T @ msg into the block's PSUM tile
    (start/stop over the block's chunks). This replaces the serialized CCE
    scatter-add entirely.
  - Finished PSUM blocks are copied to an SBUF stage (vector) and DMA'd to the
    output shard in HBM (sync engine HWDGE). Everything is ring-buffered and
    pipelined; gathers run on 2 SWDGE queues.
"""

import numpy as np

N_NODES = 100000
N_EDGES = 1250000
D = 64
NCORES = 8
SHARD = 12500
NBLK = 98              # ceil(12500/128); last block has 84 valid rows
NSEG = 4               # int16 gather indices -> x addressed in 4 segments
SEGSZ = 25000
NCELL = NBLK * NSEG
OUT_ROWS = NBLK * 128  # 12544

# pipeline depths
RC = 24      # msg ring, in cells
SEL = 64     # one-hot ring, in chunks
G = 8        # chunks per DVE compare instruction
NPS = 8      # psum tiles (one bank each)
NSTG = 4     # output stage ring, in blocks


def host_prep(x, edge_index):
    row = np.asarray(edge_index[0], dtype=np.int64)
    col = np.asarray(edge_index[1], dtype=np.int64)
    core = row // SHARD
    rloc = row - core * SHARD
    blk = rloc >> 7
    slot = rloc & 127
    seg = col // SEGSZ
    cloc = (col - seg * SEGSZ).astype(np.int16)
    cell = blk * NSEG + seg

    counts = np.zeros((NCORES, NCELL), dtype=np.int64)
    np.add.at(counts, (core, cell), 1)
    cnt_eff = np.maximum(counts, 1)          # empty cells get one dummy token
    nch = -(np.max(cnt_eff, axis=0) // -128)  # chunks per cell, shared
    cum = np.concatenate([[0], np.cumsum(nch)]).astype(np.int64)
    TC = int(cum[-1])
    T = TC * 128
    off = cum[:-1] * 128                     # token offset per cell

    import ml_dtypes
    per_core = []
    x = np.ascontiguousarray(np.asarray(x, dtype=np.float32))
    for k in range(NCORES):
        m = core == k
        ck = cell[m]
        order = np.argsort(ck, kind="stable")
        cc = ck[order]
        cl = cloc[m][order]
        sl = slot[m][order]
        cnts = np.bincount(cc, minlength=NCELL)
        starts = np.concatenate([[0], np.cumsum(cnts)])[:-1]
        within = np.arange(len(cc)) - starts[cc]
        tok = off[cc] + within
        gidx = np.full(T, -1, dtype=np.int16)
        slot_arr = np.full(T, 255, dtype=np.int32)
        gidx[tok] = cl
        slot_arr[tok] = sl
        empty = cnts == 0
        gidx[off[empty]] = 0                  # dummy valid token, slot stays 255
        gw = np.tile(gidx.reshape(-1, 16).T, (8, 1)).copy()
        sw = np.ascontiguousarray(
            slot_arr.reshape(TC, 128).T.astype(ml_dtypes.bfloat16))
        cntk = np.broadcast_to(
            np.maximum(cnts, 1).astype(np.int32), (128, NCELL)).copy()
        per_core.append({"x": x, "gidx": gw, "slot": sw, "cnt": cntk})

    return per_core, nch, cum, T, TC


def build_bass(nch, cum, T, TC):
    import concourse.bacc as bacc
    import concourse.mybir as mybir
    from concourse.bass import AP
    import contextlib

    f32, bf16, i16, i32 = (mybir.dt.float32, mybir.dt.bfloat16,
                           mybir.dt.int16, mybir.dt.int32)

    maxnch = int(np.max(nch))
    # cells: (j, seg, nch_j, first_chunk, tok0, ring_col)
    cells = []
    for j in range(NCELL):
        cells.append((j, j % NSEG, int(nch[j]), int(cum[j]), int(cum[j]) * 128,
                      (j % RC) * maxnch * 64))
    chunk_end = [int(cum[j + 1]) for j in range(NCELL)]  # chunks through cell j
    blk_chunk_end = [int(cum[(b + 1) * NSEG]) for b in range(NBLK)]

    nc = bacc.Bacc(None, target_bir_lowering=False, debug=False,
                   num_swdge_queues=4)
    x = nc.dram_tensor("x", [N_NODES, D], f32, kind="ExternalInput")
    gidx = nc.dram_tensor("gidx", [128, T // 16], i16, kind="ExternalInput")
    slot = nc.dram_tensor("slot", [128, TC], bf16, kind="ExternalInput")
    cnt = nc.dram_tensor("cnt", [128, NCELL], i32, kind="ExternalInput")
    out = nc.dram_tensor("out", [OUT_ROWS, D], f32, kind="ExternalOutput")

    last_wait = {}

    def wge(eng, sem, val):
        if val <= 0:
            return
        key = (id(eng), id(sem))
        if last_wait.get(key, 0) >= val:
            return
        eng.wait_ge(sem, val)
        last_wait[key] = val

    with (
        nc.sbuf_tensor([128, T // 16], i16) as gi_sb,
        nc.sbuf_tensor([128, TC], bf16) as slot_sb,
        nc.sbuf_tensor([128, NCELL], i32) as cnt_sb,
        nc.sbuf_tensor([128, 128], bf16) as iota_sb,
        nc.sbuf_tensor([128, RC * maxnch * 64], f32) as msg32,
        nc.sbuf_tensor([128, RC * maxnch * 64], bf16) as msg16,
        nc.sbuf_tensor([128, SEL * 128], bf16) as selT,
        nc.sbuf_tensor([128, NSTG * 64], f32) as stage,
        nc.semaphore("lsem") as lsem,
        nc.semaphore("msem") as msem,
        nc.semaphore("isem") as isem,
        nc.semaphore("csem") as csem,
        nc.semaphore("vsem") as vsem,
        nc.semaphore("pesem") as pesem,
        nc.semaphore("cpsem") as cpsem,
        contextlib.ExitStack() as stack,
        nc.Block() as block,
    ):
        # DMA completion sems rotate as deep as the consumer ring so a sem's
        # previous +16 is always consumed before its next DMA issues (the
        # sim's race detector rejects concurrent increments on one sem).
        gsems = [stack.enter_context(nc.semaphore(f"gsem{i}"))
                 for i in range(RC)]
        osems = [stack.enter_context(nc.semaphore(f"osem{i}"))
                 for i in range(NSTG)]
        psums = [stack.enter_context(nc.psum_tensor(f"ps{i}", [128, 64], f32))
                 for i in range(NPS)]

        @block.gpsimd
        def _(g):
            g.iota(iota_sb[:, :], [[1, 128]], channel_multiplier=0,
                   allow_small_or_imprecise_dtypes=True).then_inc(isem, 1)
            g.dma_start(out=gi_sb[:], in_=gidx[:]).then_inc(lsem, 16)
            g.dma_start(out=slot_sb[:], in_=slot[:]).then_inc(lsem, 16)
            g.dma_start(out=cnt_sb[:], in_=cnt[:]).then_inc(lsem, 16)
            g.wait_ge(lsem, 48)
            g.wait_ge(msem, 1)
            with g.register("gr") as gr:
                for j, s, nch_j, fc, tok0, rcol in cells:
                    wge(g, csem, j - RC + 1)
                    g.reg_load(gr, cnt_sb[0:1, j:j + 1])
                    buf = msg32[:, rcol:rcol + nch_j * 64]
                    g.dma_gather(
                        out_ap=buf.rearrange("p (k dd) -> p k dd", dd=D),
                        in_ap=x[s * SEGSZ:(s + 1) * SEGSZ, :],
                        idxs_ap=gi_sb[:, tok0 // 16:(tok0 + nch_j * 128) // 16],
                        num_idxs=nch_j * 128,
                        num_idxs_reg=gr,
                        elem_size=D,
                        single_packet=False,
                        queue_num=j % 4,
                    ).then_inc(gsems[j % RC], 16)

        @block.scalar
        def _(se):
            se.wait_ge(msem, 1)
            for j, s, nch_j, fc, tok0, rcol in cells:
                wge(se, gsems[j % RC], 16 * (j // RC + 1))
                if j >= RC:
                    wge(se, pesem, chunk_end[j - RC])
                se.copy(out=msg16[:, rcol:rcol + nch_j * 64],
                        in_=msg32[:, rcol:rcol + nch_j * 64]).then_inc(csem)

        @block.vector
        def _(ve):
            ve.memset(msg32[:], 0).then_inc(msem, 1)
            ve.wait_ge(lsem, 48)
            ve.wait_ge(isem, 1)
            ngroups = -(TC // -G)
            # merge compare groups and psum->stage copies in issue order
            events = []
            for gidx_ in range(ngroups):
                events.append((gidx_, 0, "cmp", gidx_))
            for b in range(NBLK):
                gb = (blk_chunk_end[b] - 1) // G
                events.append((gb, 1, "copy", b))
            events.sort(key=lambda e: (e[0], e[1]))
            for _, _, kind, v in events:
                if kind == "cmp":
                    g0 = v * G
                    gg = min(G, TC - g0)
                    wge(ve, pesem, g0 + gg - SEL)
                    out_ap = AP(selT, (g0 % SEL) * 128,
                                [[SEL * 128, 128], [128, gg], [1, 128]])
                    in0 = AP(slot_sb, g0,
                             [[TC, 128], [1, gg], [0, 128]])
                    in1 = AP(iota_sb, 0,
                             [[128, 128], [0, gg], [1, 128]])
                    ve.tensor_tensor(out_ap, in0, in1,
                                     mybir.AluOpType.is_equal).then_inc(vsem)
                else:
                    b = v
                    wge(ve, pesem, blk_chunk_end[b])
                    if b >= NSTG:
                        wge(ve, osems[b % NSTG], 16 * (b // NSTG))
                    ve.tensor_copy(out=stage[:, (b % NSTG) * 64:(b % NSTG + 1) * 64],
                                   in_=psums[b % NPS][:, :]).then_inc(cpsem)

        @block.tensor
        def _(te):
            for b in range(NBLK):
                first_c = blk_chunk_end[b - 1] if b > 0 else 0
                last_c = blk_chunk_end[b] - 1
                for j in range(b * NSEG, (b + 1) * NSEG):
                    _, s, nch_j, fc, tok0, rcol = cells[j]
                    wge(te, csem, j + 1)
                    for ci in range(nch_j):
                        c = fc + ci
                        wge(te, vsem, c // G + 1)
                        if c == first_c:
                            wge(te, cpsem, b - NPS + 1)
                        te.matmul(
                            psums[b % NPS][:, :],
                            selT[:, (c % SEL) * 128:(c % SEL + 1) * 128],
                            msg16[:, rcol + ci * 64:rcol + (ci + 1) * 64],
                            start=(c == first_c),
                            stop=(c == last_c),
                        ).then_inc(pesem)

        @block.sync
        def _(sy):
            for b in range(NBLK):
                wge(sy, cpsem, b + 1)
                sy.dma_start(
                    out=out[b * 128:(b + 1) * 128, :],
                    in_=stage[:, (b % NSTG) * 64:(b % NSTG + 1) * 64],
                ).then_inc(osems[b % NSTG], 16)
            for i in range(NSTG):
                n_i = NBLK // NSTG + (1 if i < NBLK % NSTG else 0)
                sy.wait_ge(osems[i], 16 * n_i)

    nc.compile()
    return nc


def run_spmd(nc, per_core, trace=False):
    from concourse.bass_utils import run_bass_kernel_spmd
    return run_bass_kernel_spmd(
        nc, per_core, core_ids=list(range(len(per_core))), trace=trace
    )


def kernel(x, edge_index, _trace=False, _return_results=False):
    x = np.asarray(x, dtype=np.float32)
    per_core, nch, cum, T, TC = host_prep(x, edge_index)
    nc = build_bass(nch, cum, T, TC)
    res = run_spmd(nc, per_core, trace=_trace)
    out = np.concatenate(
        [res.results[k]["out"][:SHARD] for k in range(NCORES)], axis=0)
    if _return_results:
        return out, res
    return out


# revision 20
# speedup vs baseline: 5.8051x; 1.2268x over previous
"""GNN message passing (scatter-add of gathered node features) on 8 TRN2 NeuronCores.

Strategy (node-sharded outputs, no collectives, no HBM read-modify-write):
  - Core k owns destination rows [k*12500, (k+1)*12500); its edges are those
    whose dest row lands in the shard. Output rows are grouped into 98 blocks
    of 128 rows.
  - Host sorts each core's edges by (block, source-segment) into fixed 128-token
    chunks; per-(block,seg) "cells" are sized by the max count over cores so all
    8 cores share one program. Padding tokens carry gather idx -1 (SWDGE skips
    trailing negatives; per-core exact counts come from a register loaded from
    an uploaded per-cell count table) and one-hot slot 255 (contributes zero).
  - Per chunk: dma_gather pulls 128 x-rows (256B each) from HBM into SBUF;
    scalar engine converts fp32->bf16; vector engine builds a [128 tok x 128
    slot] bf16 one-hot by comparing the per-token slot id against an iota row;
    PE matmul accumulates one-hot^T @ msg into the block's PSUM tile
    (start/stop over the block's chunks). This replaces the serialized CCE
    scatter-add entirely.
  - Finished PSUM blocks are copied to an SBUF stage (vector) and DMA'd to the
    output shard in HBM (sync engine HWDGE). Everything is ring-buffered and
    pipelined; gathers run on 2 SWDGE queues.
"""

import numpy as np

N_NODES = 100000
N_EDGES = 1250000
D = 64
NCORES = 8
SHARD = 12500
NBLK = 98              # ceil(12500/128); last block has 84 valid rows
NSEG = 4               # int16 gather indices -> x addressed in 4 segments
SEGSZ = 25000
NCELL = NBLK * NSEG
OUT_ROWS = NBLK * 128  # 12544

# pipeline depths
RC = 24      # msg ring, in cells
SEL = 64     # one-hot ring, in chunks
G = 8        # chunks per DVE compare instruction
NPS = 8      # psum tiles (one bank each)
NSTG = 4     # output stage ring, in blocks


def host_prep(x, edge_index):
    row = np.asarray(edge_index[0], dtype=np.int64)
    col = np.asarray(edge_index[1], dtype=np.int64)
    core = row // SHARD
    rloc = row - core * SHARD
    blk = rloc >> 7
    slot = rloc & 127
    seg = col // SEGSZ
    cloc = (col - seg * SEGSZ).astype(np.int16)
    cell = blk * NSEG + seg

    counts = np.zeros((NCORES, NCELL), dtype=np.int64)
    np.add.at(counts, (core, cell), 1)
    cnt_eff = np.maximum(counts, 1)          # empty cells get one dummy token
    nch = -(np.max(cnt_eff, axis=0) // -128)  # chunks per cell, shared
    cum = np.concatenate([[0], np.cumsum(nch)]).astype(np.int64)
    TC = int(cum[-1])
    T = TC * 128
    off = cum[:-1] * 128                     # token offset per cell

    import ml_dtypes
    per_core = []
    x = np.ascontiguousarray(np.asarray(x, dtype=np.float32))
    for k in range(NCORES):
        m = core == k
        ck = cell[m]
        order = np.argsort(ck, kind="stable")
        cc = ck[order]
        cl = cloc[m][order]
        sl = slot[m][order]
        cnts = np.bincount(cc, minlength=NCELL)
        starts = np.concatenate([[0], np.cumsum(cnts)])[:-1]
        within = np.arange(len(cc)) - starts[cc]
        tok = off[cc] + within
        # pads gather row 0 of the segment (repeated 256B read ~free in HBM);
        # their one-hot slot is 255 so they contribute zero. This beats the
        # -1-skip + per-call register count path by a lot on hardware.
        gidx = np.full(T, 0, dtype=np.int16)
        slot_arr = np.full(T, 255, dtype=np.int32)
        gidx[tok] = cl
        slot_arr[tok] = sl
        gw = np.tile(gidx.reshape(-1, 16).T, (8, 1)).copy()
        sw = np.ascontiguousarray(
            slot_arr.reshape(TC, 128).T.astype(ml_dtypes.bfloat16))
        per_core.append({"x": x, "gidx": gw, "slot": sw})

    return per_core, nch, cum, T, TC


def build_bass(nch, cum, T, TC):
    import concourse.bacc as bacc
    import concourse.mybir as mybir
    from concourse.bass import AP
    import contextlib

    f32, bf16, i16, i32 = (mybir.dt.float32, mybir.dt.bfloat16,
                           mybir.dt.int16, mybir.dt.int32)

    maxnch = int(np.max(nch))
    # cells: (j, seg, nch_j, first_chunk, tok0, ring_col)
    cells = []
    for j in range(NCELL):
        cells.append((j, j % NSEG, int(nch[j]), int(cum[j]), int(cum[j]) * 128,
                      (j % RC) * maxnch * 64))
    chunk_end = [int(cum[j + 1]) for j in range(NCELL)]  # chunks through cell j
    blk_chunk_end = [int(cum[(b + 1) * NSEG]) for b in range(NBLK)]

    nc = bacc.Bacc(None, target_bir_lowering=False, debug=False,
                   num_swdge_queues=4)
    x = nc.dram_tensor("x", [N_NODES, D], f32, kind="ExternalInput")
    gidx = nc.dram_tensor("gidx", [128, T // 16], i16, kind="ExternalInput")
    slot = nc.dram_tensor("slot", [128, TC], bf16, kind="ExternalInput")
    out = nc.dram_tensor("out", [OUT_ROWS, D], f32, kind="ExternalOutput")

    last_wait = {}

    def wge(eng, sem, val):
        if val <= 0:
            return
        key = (id(eng), id(sem))
        if last_wait.get(key, 0) >= val:
            return
        eng.wait_ge(sem, val)
        last_wait[key] = val

    with (
        nc.sbuf_tensor([128, T // 16], i16) as gi_sb,
        nc.sbuf_tensor([128, TC], bf16) as slot_sb,
        nc.sbuf_tensor([128, 128], bf16) as iota_sb,
        nc.sbuf_tensor([128, RC * maxnch * 64], f32) as msg32,
        nc.sbuf_tensor([128, RC * maxnch * 64], bf16) as msg16,
        nc.sbuf_tensor([128, SEL * 128], bf16) as selT,
        nc.sbuf_tensor([128, NSTG * 64], f32) as stage,
        nc.semaphore("lsem") as lsem,
        nc.semaphore("msem") as msem,
        nc.semaphore("isem") as isem,
        nc.semaphore("csem") as csem,
        nc.semaphore("vsem") as vsem,
        nc.semaphore("pesem") as pesem,
        nc.semaphore("cpsem") as cpsem,
        contextlib.ExitStack() as stack,
        nc.Block() as block,
    ):
        # DMA completion sems rotate as deep as the consumer ring so a sem's
        # previous +16 is always consumed before its next DMA issues (the
        # sim's race detector rejects concurrent increments on one sem).
        gsems = [stack.enter_context(nc.semaphore(f"gsem{i}"))
                 for i in range(RC)]
        osems = [stack.enter_context(nc.semaphore(f"osem{i}"))
                 for i in range(NSTG)]
        psums = [stack.enter_context(nc.psum_tensor(f"ps{i}", [128, 64], f32))
                 for i in range(NPS)]

        @block.gpsimd
        def _(g):
            g.iota(iota_sb[:, :], [[1, 128]], channel_multiplier=0,
                   allow_small_or_imprecise_dtypes=True).then_inc(isem, 1)
            g.dma_start(out=gi_sb[:], in_=gidx[:]).then_inc(lsem, 16)
            g.dma_start(out=slot_sb[:], in_=slot[:]).then_inc(lsem, 16)
            g.wait_ge(lsem, 32)
            g.wait_ge(msem, 1)
            for j, s, nch_j, fc, tok0, rcol in cells:
                wge(g, csem, j - RC + 1)
                buf = msg32[:, rcol:rcol + nch_j * 64]
                g.dma_gather(
                    out_ap=buf.rearrange("p (k dd) -> p k dd", dd=D),
                    in_ap=x[s * SEGSZ:(s + 1) * SEGSZ, :],
                    idxs_ap=gi_sb[:, tok0 // 16:(tok0 + nch_j * 128) // 16],
                    num_idxs=nch_j * 128,
                    num_idxs_reg=nch_j * 128,
                    elem_size=D,
                    single_packet=False,
                    queue_num=j % 4,
                ).then_inc(gsems[j % RC], 16)

        @block.scalar
        def _(se):
            se.wait_ge(msem, 1)
            for j, s, nch_j, fc, tok0, rcol in cells:
                wge(se, gsems[j % RC], 16 * (j // RC + 1))
                if j >= RC:
                    wge(se, pesem, chunk_end[j - RC])
                se.copy(out=msg16[:, rcol:rcol + nch_j * 64],
                        in_=msg32[:, rcol:rcol + nch_j * 64]).then_inc(csem)

        @block.vector
        def _(ve):
            ve.memset(msg32[:], 0).then_inc(msem, 1)
            ve.wait_ge(lsem, 32)
            ve.wait_ge(isem, 1)
            ngroups = -(TC // -G)
            # merge compare groups and psum->stage copies in issue order
            events = []
            for gidx_ in range(ngroups):
                events.append((gidx_, 0, "cmp", gidx_))
            for b in range(NBLK):
                gb = (blk_chunk_end[b] - 1) // G
                events.append((gb, 1, "copy", b))
            events.sort(key=lambda e: (e[0], e[1]))
            for _, _, kind, v in events:
                if kind == "cmp":
                    g0 = v * G
                    gg = min(G, TC - g0)
                    wge(ve, pesem, g0 + gg - SEL)
                    out_ap = AP(selT, (g0 % SEL) * 128,
                                [[SEL * 128, 128], [128, gg], [1, 128]])
                    in0 = AP(slot_sb, g0,
                             [[TC, 128], [1, gg], [0, 128]])
                    in1 = AP(iota_sb, 0,
                             [[128, 128], [0, gg], [1, 128]])
                    ve.tensor_tensor(out_ap, in0, in1,
                                     mybir.AluOpType.is_equal).then_inc(vsem)
                else:
                    b = v
                    wge(ve, pesem, blk_chunk_end[b])
                    if b >= NSTG:
                        wge(ve, osems[b % NSTG], 16 * (b // NSTG))
                    ve.tensor_copy(out=stage[:, (b % NSTG) * 64:(b % NSTG + 1) * 64],
                                   in_=psums[b % NPS][:, :]).then_inc(cpsem)

        @block.tensor
        def _(te):
            for b in range(NBLK):
                first_c = blk_chunk_end[b - 1] if b > 0 else 0
                last_c = blk_chunk_end[b] - 1
                for j in range(b * NSEG, (b + 1) * NSEG):
                    _, s, nch_j, fc, tok0, rcol = cells[j]
                    wge(te, csem, j + 1)
                    for ci in range(nch_j):
                        c = fc + ci
                        wge(te, vsem, c // G + 1)
                        if c == first_c:
                            wge(te, cpsem, b - NPS + 1)
                        te.matmul(
                            psums[b % NPS][:, :],
                            selT[:, (c % SEL) * 128:(c % SEL + 1) * 128],
                            msg16[:, rcol + ci * 64:rcol + (ci + 1) * 64],
                            start=(c == first_c),
                            stop=(c == last_c),
                        ).then_inc(pesem)

        @block.sync
        def _(sy):
            for b in range(NBLK):
                wge(sy, cpsem, b + 1)
                sy.dma_start(
                    out=out[b * 128:(b + 1) * 128, :],
                    in_=stage[:, (b % NSTG) * 64:(b % NSTG + 1) * 64],
                ).then_inc(osems[b % NSTG], 16)
            for i in range(NSTG):
                n_i = NBLK // NSTG + (1 if i < NBLK % NSTG else 0)
                sy.wait_ge(osems[i], 16 * n_i)

    nc.compile()
    return nc


def run_spmd(nc, per_core, trace=False):
    from concourse.bass_utils import run_bass_kernel_spmd
    return run_bass_kernel_spmd(
        nc, per_core, core_ids=list(range(len(per_core))), trace=trace
    )


def kernel(x, edge_index, _trace=False, _return_results=False):
    x = np.asarray(x, dtype=np.float32)
    per_core, nch, cum, T, TC = host_prep(x, edge_index)
    nc = build_bass(nch, cum, T, TC)
    res = run_spmd(nc, per_core, trace=_trace)
    out = np.concatenate(
        [res.results[k]["out"][:SHARD] for k in range(NCORES)], axis=0)
    if _return_results:
        return out, res
    return out


# revision 25
# speedup vs baseline: 6.0553x; 1.0431x over previous
"""GNN message passing on 8 TRN2 NeuronCores — supercell gather variant.

Like kernel.py (one-hot matmul scatter into PSUM), but the token stream is
ordered (block-group, segment, block) so ONE dma_gather covers a whole
(block-group of 4, segment) span (~16-20 chunks), cutting SWDGE call count
from 392 to 100. Converts are per-span too. PSUM accumulation groups of the
4 blocks in a group interleave (4 live PSUM tiles + pipelining).
"""

import numpy as np

N_NODES = 100000
N_EDGES = 1250000
D = 64
NCORES = 8
SHARD = 12500
NBLK = 98
NSEG = 4
SEGSZ = 25000
NCELL = NBLK * NSEG
OUT_ROWS = NBLK * 128  # 12544
BGSZ = 4               # blocks per gather group
NBG = -(NBLK // -BGSZ)  # 25

RCG = 8      # msg ring, in (bg, seg) groups (multiple of the 4 SWDGE queues)
SEL = 64     # one-hot ring, in chunks
G = 8        # chunks per DVE compare instruction
NPS = 8      # psum tiles
NSTG = 4     # output stage ring, in blocks


def host_prep(x, edge_index):
    row = np.asarray(edge_index[0], dtype=np.int64)
    col = np.asarray(edge_index[1], dtype=np.int64)
    core = row // SHARD
    rloc = row - core * SHARD
    blk = rloc >> 7
    slot = rloc & 127
    seg = col // SEGSZ
    cloc = (col - seg * SEGSZ).astype(np.int16)
    cell = blk * NSEG + seg

    counts = np.zeros((NCORES, NCELL), dtype=np.int64)
    np.add.at(counts, (core, cell), 1)
    nch = -(np.maximum(np.max(counts, axis=0), 1) // -128)  # per (b, s) cell

    # cell order: (block-group, segment, block)
    order2 = []
    for bg in range(NBG):
        for s in range(NSEG):
            for b in range(bg * BGSZ, min((bg + 1) * BGSZ, NBLK)):
                order2.append(b * NSEG + s)
    order2 = np.array(order2)
    nch2 = nch[order2]
    cum = np.concatenate([[0], np.cumsum(nch2)]).astype(np.int64)
    TC = int(cum[-1])
    T = TC * 128
    # token offset per original cell id
    off = np.zeros(NCELL, dtype=np.int64)
    off[order2] = cum[:-1] * 128

    import ml_dtypes
    per_core = []
    x = np.ascontiguousarray(np.asarray(x, dtype=np.float32))
    for k in range(NCORES):
        m = core == k
        ck = cell[m]
        okey = off[ck]
        order = np.argsort(okey, kind="stable")
        cc = ck[order]
        cl = cloc[m][order]
        sl = slot[m][order]
        cnts = np.bincount(cc, minlength=NCELL)
        # within-cell rank: cells appear contiguously in (bg, s, b) order
        starts_sorted = np.concatenate([[0], np.cumsum(cnts[order2])])[:-1]
        start_of_cell = np.zeros(NCELL, dtype=np.int64)
        start_of_cell[order2] = starts_sorted
        within = np.arange(len(cc)) - start_of_cell[cc]
        tok = off[cc] + within
        gidx = np.full(T, 0, dtype=np.int16)
        slot_arr = np.full(T, 255, dtype=np.int32)
        gidx[tok] = cl
        slot_arr[tok] = sl
        gw = np.tile(gidx.reshape(-1, 16).T, (8, 1)).copy()
        sw = np.ascontiguousarray(
            slot_arr.reshape(TC, 128).T.astype(ml_dtypes.bfloat16))
        per_core.append({"x": x, "gidx": gw, "slot": sw})

    return per_core, nch, order2, cum, T, TC


def build_bass(nch, order2, cum, T, TC):
    import concourse.bacc as bacc
    import concourse.mybir as mybir
    from concourse.bass import AP
    import contextlib

    f32, bf16, i16 = mybir.dt.float32, mybir.dt.bfloat16, mybir.dt.int16

    # groups: one per (bg, s): list of (s, first_chunk, nchunks)
    groups = []
    blk_first_chunk = {}
    blk_stop_chunk = {}
    chunk_cell_group = np.zeros(TC, dtype=np.int64)   # chunk -> group idx
    chunk_block = np.zeros(TC, dtype=np.int64)
    pos = 0
    gi = 0
    i = 0
    for bg in range(NBG):
        for s in range(NSEG):
            g_fc = pos
            for b in range(bg * BGSZ, min((bg + 1) * BGSZ, NBLK)):
                j = b * NSEG + s
                n = int(nch[j])
                assert cum[i] == pos, (i, cum[i], pos)
                chunk_cell_group[pos:pos + n] = gi
                chunk_block[pos:pos + n] = b
                if s == 0:
                    blk_first_chunk[b] = pos
                if s == NSEG - 1:
                    blk_stop_chunk[b] = pos + n - 1
                pos += n
                i += 1
            groups.append((s, g_fc, pos - g_fc))
            gi += 1
    assert pos == TC
    maxspan = max(n for _, _, n in groups)
    NGRP = len(groups)

    nc = bacc.Bacc(None, target_bir_lowering=False, debug=False,
                   num_swdge_queues=4)
    x = nc.dram_tensor("x", [N_NODES, D], f32, kind="ExternalInput")
    gidx = nc.dram_tensor("gidx", [128, T // 16], i16, kind="ExternalInput")
    slot = nc.dram_tensor("slot", [128, TC], bf16, kind="ExternalInput")
    out = nc.dram_tensor("out", [OUT_ROWS, D], f32, kind="ExternalOutput")

    last_wait = {}

    def wge(eng, sem, val):
        if val <= 0:
            return
        key = (id(eng), id(sem))
        if last_wait.get(key, 0) >= val:
            return
        eng.wait_ge(sem, val)
        last_wait[key] = val

    with (
        nc.sbuf_tensor([128, T // 16], i16) as gi_sb,
        nc.sbuf_tensor([128, TC], bf16) as slot_sb,
        nc.sbuf_tensor([128, 128], bf16) as iota_sb,
        nc.sbuf_tensor([128, RCG * maxspan * 64], f32) as msg32,
        nc.sbuf_tensor([128, RCG * maxspan * 64], bf16) as msg16,
        nc.sbuf_tensor([128, SEL * 128], bf16) as selT,
        nc.sbuf_tensor([128, NSTG * 64], f32) as stage,
        nc.semaphore("lsem") as lsem,
        nc.semaphore("msem") as msem,
        nc.semaphore("isem") as isem,
        nc.semaphore("csem") as csem,
        nc.semaphore("vsem") as vsem,
        nc.semaphore("pesem") as pesem,
        nc.semaphore("cpsem") as cpsem,
        contextlib.ExitStack() as stack,
        nc.Block() as block,
    ):
        gsems = [stack.enter_context(nc.semaphore(f"gsem{i2}"))
                 for i2 in range(RCG)]
        osems = [stack.enter_context(nc.semaphore(f"osem{i2}"))
                 for i2 in range(NSTG)]
        psums = [stack.enter_context(nc.psum_tensor(f"ps{i2}", [128, 64], f32))
                 for i2 in range(NPS)]

        # chunk -> msg ring column (group-slot base + offset within group)
        def chunk_col(c):
            g_ = chunk_cell_group[c]
            _, g_fc, _ = groups[g_]
            return (g_ % RCG) * maxspan * 64 + (c - g_fc) * 64

        @block.gpsimd
        def _(g):
            g.iota(iota_sb[:, :], [[1, 128]], channel_multiplier=0,
                   allow_small_or_imprecise_dtypes=True).then_inc(isem, 1)
            g.dma_start(out=gi_sb[:], in_=gidx[:]).then_inc(lsem, 16)
            g.dma_start(out=slot_sb[:], in_=slot[:]).then_inc(lsem, 16)
            g.wait_ge(lsem, 32)
            g.wait_ge(msem, 1)
            for gi_ in range(NGRP):
                s, g_fc, span = groups[gi_]
                wge(g, csem, gi_ - RCG + 1)
                rcol = (gi_ % RCG) * maxspan * 64
                buf = msg32[:, rcol:rcol + span * 64]
                tok0 = g_fc * 128
                g.dma_gather(
                    out_ap=buf.rearrange("p (k dd) -> p k dd", dd=D),
                    in_ap=x[s * SEGSZ:(s + 1) * SEGSZ, :],
                    idxs_ap=gi_sb[:, tok0 // 16:(tok0 + span * 128) // 16],
                    num_idxs=span * 128,
                    num_idxs_reg=span * 128,
                    elem_size=D,
                    single_packet=False,
                    queue_num=gi_ % 4,
                ).then_inc(gsems[gi_ % RCG], 16)

        @block.scalar
        def _(se):
            se.wait_ge(msem, 1)
            for gi_ in range(NGRP):
                s, g_fc, span = groups[gi_]
                wge(se, gsems[gi_ % RCG], 16 * (gi_ // RCG + 1))
                if gi_ >= RCG:
                    pg = groups[gi_ - RCG]
                    wge(se, pesem, pg[1] + pg[2])
                rcol = (gi_ % RCG) * maxspan * 64
                se.copy(out=msg16[:, rcol:rcol + span * 64],
                        in_=msg32[:, rcol:rcol + span * 64]).then_inc(csem)

        @block.vector
        def _(ve):
            ve.memset(msg32[:], 0).then_inc(msem, 1)
            ve.wait_ge(lsem, 32)
            ve.wait_ge(isem, 1)
            ngroups_c = -(TC // -G)
            events = []
            for cg in range(ngroups_c):
                events.append((cg, 0, "cmp", cg))
            for b in range(NBLK):
                gb = blk_stop_chunk[b] // G
                events.append((gb, 1, "copy", b))
            events.sort(key=lambda e: (e[0], e[1]))
            for _, _, kind, v in events:
                if kind == "cmp":
                    g0 = v * G
                    gg = min(G, TC - g0)
                    wge(ve, pesem, g0 + gg - SEL)
                    out_ap = AP(selT, (g0 % SEL) * 128,
                                [[SEL * 128, 128], [128, gg], [1, 128]])
                    in0 = AP(slot_sb, g0, [[TC, 128], [1, gg], [0, 128]])
                    in1 = AP(iota_sb, 0, [[128, 128], [0, gg], [1, 128]])
                    ve.tensor_tensor(out_ap, in0, in1,
                                     mybir.AluOpType.is_equal).then_inc(vsem)
                else:
                    b = v
                    wge(ve, pesem, blk_stop_chunk[b] + 1)
                    if b >= NSTG:
                        wge(ve, osems[b % NSTG], 16 * (b // NSTG))
                    ve.tensor_copy(out=stage[:, (b % NSTG) * 64:(b % NSTG + 1) * 64],
                                   in_=psums[b % NPS][:, :]).then_inc(cpsem)

        @block.tensor
        def _(te):
            for c in range(TC):
                b = int(chunk_block[c])
                gi_ = int(chunk_cell_group[c])
                wge(te, csem, gi_ + 1)
                wge(te, vsem, c // G + 1)
                start = (c == blk_first_chunk[b])
                if start:
                    wge(te, cpsem, b - NPS + 1)
                cc = chunk_col(c)
                te.matmul(
                    psums[b % NPS][:, :],
                    selT[:, (c % SEL) * 128:(c % SEL + 1) * 128],
                    msg16[:, cc:cc + 64],
                    start=start,
                    stop=(c == blk_stop_chunk[b]),
                    skip_group_check=True,
                ).then_inc(pesem)

        @block.sync
        def _(sy):
            for b in range(NBLK):
                wge(sy, cpsem, b + 1)
                sy.dma_start(
                    out=out[b * 128:(b + 1) * 128, :],
                    in_=stage[:, (b % NSTG) * 64:(b % NSTG + 1) * 64],
                ).then_inc(osems[b % NSTG], 16)
            for i2 in range(NSTG):
                n_i = NBLK // NSTG + (1 if i2 < NBLK % NSTG else 0)
                sy.wait_ge(osems[i2], 16 * n_i)

    nc.compile()
    return nc


def run_spmd(nc, per_core, trace=False):
    from concourse.bass_utils import run_bass_kernel_spmd
    return run_bass_kernel_spmd(
        nc, per_core, core_ids=list(range(len(per_core))), trace=trace
    )


def kernel(x, edge_index, _trace=False, _return_results=False):
    x = np.asarray(x, dtype=np.float32)
    per_core, nch, order2, cum, T, TC = host_prep(x, edge_index)
    nc = build_bass(nch, order2, cum, T, TC)
    res = run_spmd(nc, per_core, trace=_trace)
    out = np.concatenate(
        [res.results[k]["out"][:SHARD] for k in range(NCORES)], axis=0)
    if _return_results:
        return out, res
    return out


# revision 31
# speedup vs baseline: 7.5343x; 1.2442x over previous
"""GNN message passing on 8 TRN2 cores — dense-packed gather variant.

Like kernel.py, but tokens are packed DENSELY inside each (block-group of 4,
segment) span: no per-(block, segment) chunk padding (which cost ~22% extra
gather bytes). Chunks may straddle block boundaries; each (chunk, touched
block) pair becomes a matmul "variant" with its own one-hot column set (other
blocks' tokens masked to slot 255). The variant schedule is the union over
the 8 cores (all cores share one program); a variant that a core doesn't
need has an all-255 slot column there and contributes zero.
"""

import numpy as np

N_NODES = 100000
N_EDGES = 1250000
D = 64
NCORES = 8
SHARD = 12500
NBLK = 98
NSEG = 4
SEGSZ = 25000
NCELL = NBLK * NSEG
OUT_ROWS = NBLK * 128  # 12544
BGSZ = 4
NBG = -(NBLK // -BGSZ)  # 25
NGRP = NBG * NSEG       # 100 gather groups

RCG = 8      # msg ring, in groups (multiple of the 4 SWDGE queues)
SEL = 64     # one-hot ring, in variants
G = 8        # variants per DVE compare instruction
NPS = 8      # psum tiles
NSTG = 4     # output stage ring, in blocks


def host_prep(x, edge_index):
    row = np.asarray(edge_index[0], dtype=np.int64)
    col = np.asarray(edge_index[1], dtype=np.int64)
    core = row // SHARD
    rloc = row - core * SHARD
    blk = rloc >> 7
    seg = col // SEGSZ
    cloc = (col - seg * SEGSZ).astype(np.int16)
    grp = (blk // BGSZ) * NSEG + seg           # gather group id (bg, s)

    gcount = np.zeros((NCORES, NGRP), dtype=np.int64)
    np.add.at(gcount, (core, grp), 1)
    nch_g = -(np.maximum(np.max(gcount, axis=0), 1) // -128)  # chunks per group
    cum = np.concatenate([[0], np.cumsum(nch_g)]).astype(np.int64)
    TC = int(cum[-1])
    T = TC * 128
    off = cum[:-1] * 128                       # token offset per group

    # per-core token streams, densely packed, sorted by (group, block, rloc)
    import ml_dtypes
    x = np.ascontiguousarray(np.asarray(x, dtype=np.float32))
    core_tok_rloc = []                         # per core: rloc per token (pad=BIG)
    gws = []
    BIG = 1 << 30
    for k in range(NCORES):
        m = core == k
        gk = grp[m]
        order = np.lexsort((rloc[m], gk))
        gk = gk[order]
        clk = cloc[m][order]
        rlk = rloc[m][order]
        cnts = np.bincount(gk, minlength=NGRP)
        starts = np.concatenate([[0], np.cumsum(cnts)])[:-1]
        within = np.arange(len(gk)) - starts[gk]
        tok = off[gk] + within
        gidx = np.full(T, 0, dtype=np.int16)   # pads gather row 0
        rl = np.full(T, BIG, dtype=np.int64)
        gidx[tok] = clk
        rl[tok] = rlk
        gws.append(np.tile(gidx.reshape(-1, 16).T, (8, 1)).copy())
        core_tok_rloc.append(rl)

    # variant schedule: union over cores of blocks present per chunk
    bmin = np.full(TC, 1 << 20, dtype=np.int64)
    bmax = np.full(TC, -1, dtype=np.int64)
    for k in range(NCORES):
        bt = core_tok_rloc[k] >> 7             # block per token (pads huge)
        btc = bt.reshape(TC, 128)
        valid = btc < NBLK
        btc_min = np.where(valid, btc, 1 << 20).min(axis=1)
        btc_max = np.where(valid, btc, -1).max(axis=1)
        bmin = np.minimum(bmin, btc_min)
        bmax = np.maximum(bmax, btc_max)

    variants = []                              # (chunk, block)
    for c in range(TC):
        if bmax[c] >= 0:
            for b in range(int(bmin[c]), int(bmax[c]) + 1):
                variants.append((c, b))
    # ensure every block has at least one variant
    have = {b for _, b in variants}
    for b in range(NBLK):
        if b not in have:
            g0 = (b // BGSZ) * NSEG            # its (bg, s=0) group
            variants.append((int(cum[g0]), b))
    variants.sort()
    var_c = np.array([c for c, _ in variants], dtype=np.int64)
    var_b = np.array([b for _, b in variants], dtype=np.int64)
    NV = len(variants)

    # per-core slot table [128, NV]: token p of chunk var_c relative to var_b
    per_core = []
    for k in range(NCORES):
        rl = core_tok_rloc[k].reshape(TC, 128)
        sv = rl[var_c] - var_b[:, None] * 128  # [NV, 128]
        sv = np.where((sv >= 0) & (sv < 128), sv, 255).astype(np.int32)
        sw = np.ascontiguousarray(sv.T.astype(ml_dtypes.bfloat16))
        per_core.append({"x": x, "gidx": gws[k], "slot": sw})

    return per_core, nch_g, cum, var_c, var_b, T, TC, NV


def build_bass(nch_g, cum, var_c, var_b, T, TC, NV):
    import concourse.bacc as bacc
    import concourse.mybir as mybir
    from concourse.bass import AP
    import contextlib

    f32, bf16, i16 = mybir.dt.float32, mybir.dt.bfloat16, mybir.dt.int16

    maxspan = int(np.max(nch_g))
    chunk_grp = np.zeros(TC, dtype=np.int64)
    for g_ in range(NGRP):
        chunk_grp[cum[g_]:cum[g_ + 1]] = g_

    # per block: first/last variant index; per group: last variant index
    blk_first_v = {}
    blk_last_v = {}
    for v in range(NV):
        b = int(var_b[v])
        if b not in blk_first_v:
            blk_first_v[b] = v
        blk_last_v[b] = v
    grp_last_v = np.full(NGRP, -1, dtype=np.int64)
    for v in range(NV):
        grp_last_v[chunk_grp[var_c[v]]] = v

    nc = bacc.Bacc(None, target_bir_lowering=False, debug=False,
                   num_swdge_queues=4)
    x = nc.dram_tensor("x", [N_NODES, D], f32, kind="ExternalInput")
    gidx = nc.dram_tensor("gidx", [128, T // 16], i16, kind="ExternalInput")
    slot = nc.dram_tensor("slot", [128, NV], bf16, kind="ExternalInput")
    out = nc.dram_tensor("out", [OUT_ROWS, D], f32, kind="ExternalOutput")

    last_wait = {}

    def wge(eng, sem, val):
        if val <= 0:
            return
        key = (id(eng), id(sem))
        if last_wait.get(key, 0) >= val:
            return
        eng.wait_ge(sem, val)
        last_wait[key] = val

    with (
        nc.sbuf_tensor([128, T // 16], i16) as gi_sb,
        nc.sbuf_tensor([128, NV], bf16) as slot_sb,
        nc.sbuf_tensor([128, 128], bf16) as iota_sb,
        nc.sbuf_tensor([128, RCG * maxspan * 64], f32) as msg32,
        nc.sbuf_tensor([128, RCG * maxspan * 64], bf16) as msg16,
        nc.sbuf_tensor([128, SEL * 128], bf16) as selT,
        nc.sbuf_tensor([128, NSTG * 64], f32) as stage,
        nc.semaphore("lsem") as lsem,
        nc.semaphore("lsemB") as lsemB,
        nc.semaphore("lsemC") as lsemC,
        nc.semaphore("msem") as msem,
        nc.semaphore("isem") as isem,
        nc.semaphore("csem") as csem,
        nc.semaphore("vsem") as vsem,
        nc.semaphore("pesem") as pesem,
        nc.semaphore("cpsem") as cpsem,
        contextlib.ExitStack() as stack,
        nc.Block(no_gpsimd_drain=True) as block,
    ):
        gsems = [stack.enter_context(nc.semaphore(f"gsem{i2}"))
                 for i2 in range(RCG)]
        osems = [stack.enter_context(nc.semaphore(f"osem{i2}"))
                 for i2 in range(NSTG)]
        psums = [stack.enter_context(nc.psum_tensor(f"ps{i2}", [128, 64], f32))
                 for i2 in range(NPS)]

        def chunk_col(c):
            g_ = int(chunk_grp[c])
            return (g_ % RCG) * maxspan * 64 + (c - int(cum[g_])) * 64

        # first slice: exactly what the first ring-fill of gathers needs
        IH0 = (int(cum[min(RCG, NGRP)]) * 128) // 16
        IH = max(IH0 + 1, (T // 16) // 2)

        @block.sync
        def _(sy):
            sy.dma_start(out=gi_sb[:, :IH0], in_=gidx[:, :IH0]).then_inc(lsem, 16)
            sy.dma_start(out=gi_sb[:, IH0:IH], in_=gidx[:, IH0:IH]).then_inc(lsemB, 16)
            sy.dma_start(out=gi_sb[:, IH:], in_=gidx[:, IH:]).then_inc(lsemC, 16)
            sy.dma_start(out=slot_sb[:], in_=slot[:]).then_inc(isem, 16)

        @block.gpsimd
        def _(g):
            g.iota(iota_sb[:, :], [[1, 128]], channel_multiplier=0,
                   allow_small_or_imprecise_dtypes=True).then_inc(isem, 1)
            g.wait_ge(msem, 1)
            for gi_ in range(NGRP):
                s = gi_ % NSEG
                span = int(nch_g[gi_])
                tokend = (int(cum[gi_]) + span) * 128
                if tokend // 16 <= IH0:
                    wge(g, lsem, 16)
                elif tokend // 16 <= IH:
                    wge(g, lsemB, 16)
                else:
                    wge(g, lsemC, 16)
                wge(g, csem, gi_ - RCG + 1)
                rcol = (gi_ % RCG) * maxspan * 64
                buf = msg32[:, rcol:rcol + span * 64]
                tok0 = int(cum[gi_]) * 128
                g.dma_gather(
                    out_ap=buf.rearrange("p (k dd) -> p k dd", dd=D),
                    in_ap=x[s * SEGSZ:(s + 1) * SEGSZ, :],
                    idxs_ap=gi_sb[:, tok0 // 16:(tok0 + span * 128) // 16],
                    num_idxs=span * 128,
                    num_idxs_reg=span * 128,
                    elem_size=D,
                    single_packet=False,
                    queue_num=gi_ % 4,
                ).then_inc(gsems[gi_ % RCG], 16)

        @block.scalar
        def _(se):
            se.wait_ge(msem, 1)
            for gi_ in range(NGRP):
                span = int(nch_g[gi_])
                wge(se, gsems[gi_ % RCG], 16 * (gi_ // RCG + 1))
                if gi_ >= RCG:
                    wge(se, pesem, int(grp_last_v[gi_ - RCG]) + 1)
                rcol = (gi_ % RCG) * maxspan * 64
                se.copy(out=msg16[:, rcol:rcol + span * 64],
                        in_=msg32[:, rcol:rcol + span * 64]).then_inc(csem)

        @block.vector
        def _(ve):
            ve.memset(msg32[:], 0).then_inc(msem, 1)
            ve.wait_ge(isem, 17)
            nvg = -(NV // -G)
            events = []
            for vg in range(nvg):
                events.append((vg, 0, "cmp", vg))
            for b in range(NBLK):
                gb = blk_last_v[b] // G
                events.append((gb, 1, "copy", b))
            events.sort(key=lambda e: (e[0], e[1]))
            for _, _, kind, v in events:
                if kind == "cmp":
                    v0 = v * G
                    gg = min(G, NV - v0)
                    wge(ve, pesem, v0 + gg - SEL)
                    out_ap = AP(selT, (v0 % SEL) * 128,
                                [[SEL * 128, 128], [128, gg], [1, 128]])
                    in0 = AP(slot_sb, v0, [[NV, 128], [1, gg], [0, 128]])
                    in1 = AP(iota_sb, 0, [[128, 128], [0, gg], [1, 128]])
                    ve.tensor_tensor(out_ap, in0, in1,
                                     mybir.AluOpType.is_equal).then_inc(vsem)
                else:
                    b = v
                    wge(ve, pesem, blk_last_v[b] + 1)
                    if b >= NSTG:
                        wge(ve, osems[b % NSTG], 16 * (b // NSTG))
                    ve.tensor_copy(out=stage[:, (b % NSTG) * 64:(b % NSTG + 1) * 64],
                                   in_=psums[b % NPS][:, :]).then_inc(cpsem)

        @block.tensor
        def _(te):
            for v in range(NV):
                c = int(var_c[v])
                b = int(var_b[v])
                wge(te, csem, int(chunk_grp[c]) + 1)
                wge(te, vsem, v // G + 1)
                start = (v == blk_first_v[b])
                if start:
                    wge(te, cpsem, b - NPS + 1)
                cc = chunk_col(c)
                te.matmul(
                    psums[b % NPS][:, :],
                    selT[:, (v % SEL) * 128:(v % SEL + 1) * 128],
                    msg16[:, cc:cc + 64],
                    start=start,
                    stop=(v == blk_last_v[b]),
                    skip_group_check=True,
                ).then_inc(pesem)

        @block.sync
        def _(sy):
            for b in range(NBLK):
                wge(sy, cpsem, b + 1)
                sy.dma_start(
                    out=out[b * 128:(b + 1) * 128, :],
                    in_=stage[:, (b % NSTG) * 64:(b % NSTG + 1) * 64],
                ).then_inc(osems[b % NSTG], 16)
            for i2 in range(NSTG):
                n_i = NBLK // NSTG + (1 if i2 < NBLK % NSTG else 0)
                sy.wait_ge(osems[i2], 16 * n_i)

    nc.compile()
    return nc


def run_spmd(nc, per_core, trace=False):
    from concourse.bass_utils import run_bass_kernel_spmd
    return run_bass_kernel_spmd(
        nc, per_core, core_ids=list(range(len(per_core))), trace=trace
    )


def kernel(x, edge_index, _trace=False, _return_results=False):
    x = np.asarray(x, dtype=np.float32)
    per_core, nch_g, cum, var_c, var_b, T, TC, NV = host_prep(x, edge_index)
    nc = build_bass(nch_g, cum, var_c, var_b, T, TC, NV)
    res = run_spmd(nc, per_core, trace=_trace)
    out = np.concatenate(
        [res.results[k]["out"][:SHARD] for k in range(NCORES)], axis=0)
    if _return_results:
        return out, res
    return out
